# revision 10
# baseline (speedup 1.0000x reference)
"""BackboneHBonds TRN2 kernel.

Sharding: residue axis N split across 8 NeuronCores (2048 rows/core, all 4
batches per core). Neighbor gather = InstDMAGatherAnt (256B rows from a
per-batch DRAM table) spread over 4 SWDGE queues, 16 in flight. Per-edge
math on DVE (custom fused ops) + ACT (sqrt); 1/d via sqrt + fast-reciprocal
+ one Newton rsqrt step (<=2 ulp).

Host side only reshapes/casts/shards inputs and re-assembles outputs.
"""
import numpy as np

import concourse.bass as bass
import concourse.bacc as bacc
import concourse.tile as tile
import concourse.mybir as mybir
from concourse.bass_utils import run_bass_kernel_spmd
from concourse.dve_spec import (
    Spec, Src0, Src1, C0, C1, C2, One, sq, eq, lower,
)
from concourse.dve_ops import has_src1
from concourse.dve_ops import (
    DveOp, OPS, _SUB_OPCODE_FOR_NAME, CUSTOM_DVE_SPECS, RECIPROCAL_APPROX_FAST,
)
from concourse.dve_uop import DveOpSpec

B, N, K = 4, 16384, 64
NC = 8
R = N // NC          # 2048 rows per core
BLK = 16             # row blocks of 128 per core per batch
P = 128
NI = 2048            # edges per gather chunk
E = 64               # f32 per table row (256B)
NUNITS = B * BLK     # 64 compute units per core, 4 chunks each
EPS = np.float32(1e-3)
GAMMA = np.float32(0.42 * 0.2 * 332.0)
LEN_NH = np.float32(1.015)


def _f32_sqrt_lt_threshold(cut=np.float32(3.6)):
    # exact f32 boundary: smallest x with fl(sqrt(x)) >= cut, so that
    # (d2 < thr) == (np.sqrt(d2) < cut) elementwise in f32
    x = np.float32(cut) * np.float32(cut)
    while np.sqrt(np.float32(x), dtype=np.float32) >= cut:
        x = np.nextafter(x, np.float32(0), dtype=np.float32)
    while np.sqrt(np.float32(x), dtype=np.float32) < cut:
        x = np.nextafter(x, np.float32(np.inf), dtype=np.float32)
    return np.float32(x)  # first x whose sqrt >= cut

THR_D2 = _f32_sqrt_lt_threshold()

_REGISTERED = {}


def _register(name, body, reference):
    if name in _REGISTERED:
        return _REGISTERED[name]
    spec = Spec(body=body, reference=reference)
    row = max(_SUB_OPCODE_FOR_NAME.values()) + 1
    assert row < 0x20
    shas = {}
    for ver in ("v3", "v4"):
        tmp = DveOpSpec(name=name, opcode=row, uops=lower(spec, ver=ver),
                        rd1_en=has_src1(spec))
        shas[ver] = tmp.sha(ver)
    op = DveOp(name, spec, subdim=False, uops_sha=shas)
    OPS.append(op)
    _SUB_OPCODE_FOR_NAME[name] = row
    CUSTOM_DVE_SPECS[name] = spec
    _REGISTERED[name] = op
    return op


OPA = _register(
    "HB_SQ2", sq(Src0 - C0) + sq(Src1 - C1),
    lambda in0, in1, s0, s1, imm2: (in0 - s0) ** 2 + (in1 - s1) ** 2)
OPB = _register(
    "HB_SQ3E", (Src0 + sq(Src1 - C0)) + C2,
    lambda in0, in1, s0, s1, imm2: in0 + (in1 - s0) ** 2 + imm2)
OPNR = _register(
    "HB_NR", Src1 * (C0 - (Src0 * C2) * sq(Src1)),
    lambda in0, in1, s0, s1, imm2: in1 * (s0 - (in0 * imm2) * in1 * in1))
OPU = _register(
    "HB_UCMP", ((Src0 + Src1) * C0) < C1,
    lambda in0, in1, s0, s1, imm2: (((in0 + in1) * s0) < s1).astype(np.float32))
OPL1 = _register(
    "HB_BAND", ((Src0 > C0) & (Src0 < C1)) * Src1,
    lambda in0, in1, s0, s1, imm2: (((in0 > s0) & (in0 < s1)) * in1).astype(np.float32))
OPL2 = _register(
    "HB_NLOC", (One - eq(Src0, C0)) * Src1,
    lambda in0, in1, s0, s1, imm2: ((in0 != s0) * in1).astype(np.float32))


def build_nc():
    nc = bacc.Bacc("TRN2", target_bir_lowering=False, num_swdge_queues=4)
    f32 = mybir.dt.float32

    tab = nc.dram_tensor("tab", [B * N, E], f32, kind="ExternalInput")
    idxw_d = nc.dram_tensor("idxw", [P, NUNITS * NI // 16], mybir.dt.int16, kind="ExternalInput")
    jf_d = nc.dram_tensor("jf", [NUNITS, P, K], f32, kind="ExternalInput")
    mk_d = nc.dram_tensor("mk", [NUNITS, P, K], f32, kind="ExternalInput")
    xr_d = nc.dram_tensor("xr", [B, P, BLK * 9], f32, kind="ExternalInput")
    ci_d = nc.dram_tensor("ci", [B, P, BLK], f32, kind="ExternalInput")
    io_d = nc.dram_tensor("io", [P, BLK], f32, kind="ExternalInput")

    hb_d = nc.dram_tensor("hb", [NUNITS, P, K], f32, kind="ExternalOutput")
    mh_d = nc.dram_tensor("mh", [NUNITS, P, K], f32, kind="ExternalOutput")
    ho_d = nc.dram_tensor("ho", [B, P, BLK * 3], f32, kind="ExternalOutput")

    g_sems = [nc.alloc_semaphore(f"g_sem{q}") for q in range(4)]
    SQRT = mybir.ActivationFunctionType.Sqrt
    ni_reg = nc.gpsimd.to_reg(NI)  # preamble: POOL executes this before Tile region

    def rsqrt_refined(pool, tag, d2_ap, w):
        # y ~= 1/sqrt(d2_ap) to ~2ulp: ACT sqrt seed -> recip approx -> 1 NR
        s = pool.tile([P, w], mybir.dt.float32, tag=tag + "_s")
        nc.scalar.activation(s[:], d2_ap, SQRT)
        y0 = pool.tile([P, w], mybir.dt.float32, tag=tag + "_y0")
        nc.vector.reciprocal_approx_fast(y0[:], s[:])
        y1 = pool.tile([P, w], mybir.dt.float32, tag=tag + "_y1")
        nc.vector._custom_dve(OPNR, out=y1[:], in0=d2_ap, in1=y0[:],
                              s0=1.5, imm2=0.5)
        return y1

    with tile.TileContext(nc) as tc:
        with tc.tile_pool(name="stat", bufs=1) as sp, \
             tc.tile_pool(name="ring", bufs=3) as rp, \
             tc.tile_pool(name="tmp", bufs=3) as tp:

            idxw = sp.tile([P, NUNITS * NI // 16], mybir.dt.int16)
            nc.sync.dma_start(out=idxw[:], in_=idxw_d[:])
            io_t = sp.tile([P, BLK], mybir.dt.float32)
            nc.sync.dma_start(out=io_t[:], in_=io_d[:])
            lo_t = sp.tile([P, BLK], mybir.dt.float32)
            hi_t = sp.tile([P, BLK], mybir.dt.float32)
            nc.vector.tensor_scalar(out=lo_t[:], in0=io_t[:], scalar1=-2.5,
                                    scalar2=None, op0=mybir.AluOpType.add)
            nc.vector.tensor_scalar(out=hi_t[:], in0=io_t[:], scalar1=2.5,
                                    scalar2=None, op0=mybir.AluOpType.add)

            xr_t, ci_t, mi_t, ht = [], [], [], []
            for b in range(B):
                x = sp.tile([P, BLK * 9], mybir.dt.float32)
                nc.sync.dma_start(out=x[:], in_=xr_d[b])
                xr_t.append(x)
                c = sp.tile([P, BLK], mybir.dt.float32)
                nc.sync.dma_start(out=c[:], in_=ci_d[b])
                ci_t.append(c)

            # ---- prologue per batch: H_i and mask_i ----
            for b in range(B):
                x3 = xr_t[b][:].rearrange("p (l c) -> p l c", c=9)
                nv = x3[:, :, 0:3]
                u1 = sp.tile([P, BLK * 3], mybir.dt.float32, tag=f"u1_{b}")
                u2 = sp.tile([P, BLK * 3], mybir.dt.float32, tag=f"u2_{b}")
                u1v = u1[:].rearrange("p (l c) -> p l c", c=3)
                u2v = u2[:].rearrange("p (l c) -> p l c", c=3)
                nc.vector.tensor_tensor(out=u1v, in0=nv, in1=x3[:, :, 6:9],
                                        op=mybir.AluOpType.subtract)
                nc.vector.tensor_tensor(out=u2v, in0=nv, in1=x3[:, :, 3:6],
                                        op=mybir.AluOpType.subtract)
                # norms^2 packed [P, 2*BLK]: cols 0:BLK = |u1|^2, BLK: = |u2|^2
                n2 = sp.tile([P, 2 * BLK], mybir.dt.float32, tag=f"n2_{b}")
                for ui, uv, off in ((0, u1v, 0), (1, u2v, BLK)):
                    sqt = tp.tile([P, BLK * 3], mybir.dt.float32, tag="psq")
                    sqv = sqt[:].rearrange("p (l c) -> p l c", c=3)
                    nc.vector.tensor_tensor(out=sqv, in0=uv, in1=uv,
                                            op=mybir.AluOpType.mult)
                    acc = tp.tile([P, BLK], mybir.dt.float32, tag="pacc")
                    nc.vector.tensor_tensor(out=acc[:], in0=sqv[:, :, 0],
                                            in1=sqv[:, :, 1], op=mybir.AluOpType.add)
                    nc.vector.tensor_tensor(out=acc[:], in0=acc[:],
                                            in1=sqv[:, :, 2], op=mybir.AluOpType.add)
                    nc.vector.tensor_scalar(out=n2[:, off:off + BLK], in0=acc[:],
                                            scalar1=float(EPS), scalar2=None,
                                            op0=mybir.AluOpType.add)
                y12 = rsqrt_refined(tp, "pro", n2[:], 2 * BLK)
                # w = u1*y1 + u2*y2 (per comp), then normalize, H = N + 1.015*w_hat
                wt = sp.tile([P, BLK * 3], mybir.dt.float32, tag=f"w_{b}")
                wv = wt[:].rearrange("p (l c) -> p l c", c=3)
                for cc in range(3):
                    t1 = tp.tile([P, BLK], mybir.dt.float32, tag="pt1")
                    nc.vector.tensor_tensor(out=t1[:], in0=u1v[:, :, cc],
                                            in1=y12[:, 0:BLK], op=mybir.AluOpType.mult)
                    t2 = tp.tile([P, BLK], mybir.dt.float32, tag="pt2")
                    nc.vector.tensor_tensor(out=t2[:], in0=u2v[:, :, cc],
                                            in1=y12[:, BLK:], op=mybir.AluOpType.mult)
                    nc.vector.tensor_tensor(out=wv[:, :, cc], in0=t1[:], in1=t2[:],
                                            op=mybir.AluOpType.add)
                nw = tp.tile([P, BLK], mybir.dt.float32, tag="pnw")
                sqw = tp.tile([P, BLK * 3], mybir.dt.float32, tag="psqw")
                sqwv = sqw[:].rearrange("p (l c) -> p l c", c=3)
                nc.vector.tensor_tensor(out=sqwv, in0=wv, in1=wv,
                                        op=mybir.AluOpType.mult)
                nc.vector.tensor_tensor(out=nw[:], in0=sqwv[:, :, 0],
                                        in1=sqwv[:, :, 1], op=mybir.AluOpType.add)
                nc.vector.tensor_tensor(out=nw[:], in0=nw[:], in1=sqwv[:, :, 2],
                                        op=mybir.AluOpType.add)
                nc.vector.tensor_scalar(out=nw[:], in0=nw[:], scalar1=float(EPS),
                                        scalar2=None, op0=mybir.AluOpType.add)
                yw = rsqrt_refined(tp, "prw", nw[:], BLK)
                h = sp.tile([P, BLK * 3], mybir.dt.float32, tag=f"h_{b}")
                hv = h[:].rearrange("p (l c) -> p l c", c=3)
                for cc in range(3):
                    t1 = tp.tile([P, BLK], mybir.dt.float32, tag="pt3")
                    nc.vector.tensor_tensor(out=t1[:], in0=wv[:, :, cc], in1=yw[:],
                                            op=mybir.AluOpType.mult)
                    nc.vector.tensor_scalar(out=t1[:], in0=t1[:],
                                            scalar1=float(LEN_NH), scalar2=None,
                                            op0=mybir.AluOpType.mult)
                    nc.vector.tensor_tensor(out=hv[:, :, cc], in0=t1[:],
                                            in1=x3[:, :, cc], op=mybir.AluOpType.add)
                ht.append(h)
                nc.sync.dma_start(out=ho_d[b], in_=h[:])
                m = sp.tile([P, BLK], mybir.dt.float32, tag=f"mi_{b}")
                nc.vector.tensor_scalar(out=m[:], in0=ci_t[b][:], scalar1=0.0,
                                        scalar2=None, op0=mybir.AluOpType.is_gt)
                mi_t.append(m)

            # ---- main loop over units ----
            for u in range(NUNITS):
                b, blk = divmod(u, BLK)
                et = rp.tile([P, K * E], mybir.dt.float32, tag="et")
                etv = et[:].rearrange("p (s c) -> p s c", c=E)
                for qq in range(4):
                    ug = u * 4 + qq
                    q, cb = ug % 4, ug // 4
                    nc.gpsimd.dma_gather(
                        out_ap=etv[:, qq * 16:(qq + 1) * 16, :],
                        in_ap=tab[b * N:(b + 1) * N, :],
                        idxs_ap=idxw[:, cb * 128:(cb + 1) * 128],
                        num_idxs=NI, num_idxs_reg=ni_reg, elem_size=E,
                        single_packet=False, queue_num=q,
                    ).then_inc(g_sems[q], 16)
                wn = 16 * (u + 1)
                for q in range(4):
                    cr = et[:].rearrange("p (s c) -> p s c", c=E)[:, q * 16:(q + 1) * 16, 0:8]
                    nc.scalar.copy(cr, cr)._wait_ge(g_sems[q], wn)

                def gwait(inst):
                    return inst

                jt = rp.tile([P, K], mybir.dt.float32, tag="jt")
                nc.sync.dma_start(out=jt[:], in_=jf_d[u])
                mt = rp.tile([P, K], mybir.dt.float32, tag="mt")
                nc.sync.dma_start(out=mt[:], in_=mk_d[u])

                comp = et[:].rearrange("p (s c) -> p c s", c=E)
                Cx, Cy, Cz = comp[:, 0], comp[:, 1], comp[:, 2]
                Ox, Oy, Oz = comp[:, 3], comp[:, 4], comp[:, 5]
                Cg = comp[:, 6]
                xr3 = xr_t[b][:].rearrange("p (l c) -> p l c", c=9)
                Nx, Ny, Nz = (xr3[:, blk, cc:cc + 1] for cc in range(3))
                hv = ht[b][:].rearrange("p (l c) -> p l c", c=3)
                Hx, Hy, Hz = (hv[:, blk, cc:cc + 1] for cc in range(3))

                d2 = {}
                for nm, (bx, by, bz), (ax, ay, az) in (
                        ("NO", (Ox, Oy, Oz), (Nx, Ny, Nz)),
                        ("NC", (Cx, Cy, Cz), (Nx, Ny, Nz)),
                        ("HC", (Cx, Cy, Cz), (Hx, Hy, Hz)),
                        ("HO", (Ox, Oy, Oz), (Hx, Hy, Hz))):
                    c1 = tp.tile([P, K], mybir.dt.float32, tag=f"c1{nm}")
                    gwait(nc.vector._custom_dve(OPA, out=c1[:], in0=bx, in1=by,
                                          s0=ax, s1=ay))
                    dd = tp.tile([P, K], mybir.dt.float32, tag=f"d2{nm}")
                    gwait(nc.vector._custom_dve(OPB, out=dd[:], in0=c1[:], in1=bz,
                                          s0=az, imm2=float(EPS)))
                    d2[nm] = dd
                y1 = {nm: rsqrt_refined(tp, f"e{nm}", d2[nm][:], K) for nm in d2}

                u1t = tp.tile([P, K], mybir.dt.float32, tag="u1t")
                nc.vector.tensor_tensor(out=u1t[:], in0=y1["NO"][:], in1=y1["NC"][:],
                                        op=mybir.AluOpType.subtract)
                u2t = tp.tile([P, K], mybir.dt.float32, tag="u2t")
                nc.vector.tensor_tensor(out=u2t[:], in0=y1["HC"][:], in1=y1["HO"][:],
                                        op=mybir.AluOpType.subtract)
                ucmp = tp.tile([P, K], mybir.dt.float32, tag="ucmp")
                nc.vector._custom_dve(OPU, out=ucmp[:], in0=u1t[:], in1=u2t[:],
                                      s0=float(GAMMA), s1=-0.5)
                cutd = tp.tile([P, K], mybir.dt.float32, tag="cutd")
                nc.vector.tensor_scalar(out=cutd[:], in0=d2["NO"][:],
                                        scalar1=float(THR_D2), scalar2=None,
                                        op0=mybir.AluOpType.is_lt)
                t1m = tp.tile([P, K], mybir.dt.float32, tag="t1m")
                gwait(nc.vector._custom_dve(OPL1, out=t1m[:], in0=jt[:], in1=Cg,
                                      s0=lo_t[:, blk:blk + 1], s1=hi_t[:, blk:blk + 1]))
                nlm = tp.tile([P, K], mybir.dt.float32, tag="nlm")
                nc.vector._custom_dve(OPL2, out=nlm[:], in0=t1m[:], in1=mt[:],
                                      s0=ci_t[b][:, blk:blk + 1])
                cgp = tp.tile([P, K], mybir.dt.float32, tag="cgp")
                gwait(nc.vector.tensor_scalar(out=cgp[:], in0=Cg, scalar1=0.0,
                                        scalar2=None, op0=mybir.AluOpType.is_gt))
                m1 = tp.tile([P, K], mybir.dt.float32, tag="m1")
                nc.vector.tensor_tensor(out=m1[:], in0=nlm[:], in1=cutd[:],
                                        op=mybir.AluOpType.mult)
                nc.vector.tensor_tensor(out=m1[:], in0=m1[:], in1=cgp[:],
                                        op=mybir.AluOpType.mult)
                mhv = rp.tile([P, K], mybir.dt.float32, tag="mhv")
                nc.vector.tensor_scalar(out=mhv[:], in0=m1[:],
                                        scalar1=mi_t[b][:, blk:blk + 1], scalar2=None,
                                        op0=mybir.AluOpType.mult)
                hbv = rp.tile([P, K], mybir.dt.float32, tag="hbv")
                nc.vector.tensor_tensor(out=hbv[:], in0=mhv[:], in1=ucmp[:],
                                        op=mybir.AluOpType.mult)
                nc.sync.dma_start(out=mh_d[u], in_=mhv[:])
                nc.sync.dma_start(out=hb_d[u], in_=hbv[:])

    return nc


_NC_CACHE = None


def _get_nc():
    global _NC_CACHE
    if _NC_CACHE is None:
        nc = build_nc()
        nc.compile()
        nc.finalize()
        _NC_CACHE = nc
    return _NC_CACHE


def kernel(X, C, edge_idx, mask_ij):
    X = np.asarray(X, dtype=np.float32)
    C64 = np.asarray(C)
    e64 = np.asarray(edge_idx)
    mask_ij = np.asarray(mask_ij, dtype=np.float32)

    # layout-only host prep: gather table rows, index repack, shards
    tab = np.zeros((B, N, E), np.float32)
    tab[:, :, 0:3] = X[:, :, 2, :]
    tab[:, :, 3:6] = X[:, :, 3, :]
    tab[:, :, 6] = C64.astype(np.float32)
    tab = tab.reshape(B * N, E)
    Xprev = np.concatenate([X[:, :1], X[:, :-1]], axis=1)[:, :, 2, :]  # (B,N,3)

    in_maps = []
    for c in range(NC):
        rows = slice(c * R, (c + 1) * R)
        ej = e64[:, rows].astype(np.int32).reshape(B, BLK, P, K)
        idxw = np.zeros((P, NUNITS * NI // 16), np.int16)
        for u in range(NUNITS):
            b, blk = divmod(u, BLK)
            for qq in range(4):
                ug = u * 4 + qq
                q, cb = ug % 4, ug // 4
                flat = ej[b, blk, :, 16 * qq:16 * (qq + 1)].T.ravel()  # i=kk*128+p
                w = flat.reshape(NI // 16, 16).T.astype(np.int16)      # [16, 128]
                idxw[32 * q:32 * q + 16, cb * 128:(cb + 1) * 128] = w
                idxw[32 * q + 16:32 * q + 32, cb * 128:(cb + 1) * 128] = w
        jf = ej.astype(np.float32).reshape(NUNITS, P, K)
        mk = mask_ij[:, rows].reshape(NUNITS, P, K)
        xn = X[:, rows].reshape(B, BLK, P, 4, 3)
        xp = Xprev[:, rows].reshape(B, BLK, P, 3)
        xr = np.concatenate([xn[:, :, :, 0, :], xn[:, :, :, 1, :], xp], axis=-1)
        xr = xr.transpose(0, 2, 1, 3).reshape(B, P, BLK * 9).astype(np.float32)
        ci = C64[:, rows].reshape(B, BLK, P).transpose(0, 2, 1).astype(np.float32)
        io = np.arange(c * R, (c + 1) * R, dtype=np.float32).reshape(BLK, P).T.copy()
        in_maps.append({"tab": tab, "idxw": idxw, "jf": jf, "mk": mk,
                        "xr": xr, "ci": ci, "io": io})

    res = run_bass_kernel_spmd(_get_nc(), in_maps, core_ids=list(range(NC)))

    hbonds = np.zeros((B, N, K), np.float32)
    mask_hb = np.zeros((B, N, K), np.float32)
    H = np.zeros((B, N, 3), np.float32)
    for c in range(NC):
        rows = slice(c * R, (c + 1) * R)
        r = res.results[c]
        hbonds[:, rows] = r["hb"].reshape(B, BLK * P, K)
        mask_hb[:, rows] = r["mh"].reshape(B, BLK * P, K)
        H[:, rows] = (r["ho"].reshape(B, P, BLK, 3)
                      .transpose(0, 2, 1, 3).reshape(B, R, 3))
    return hbonds, mask_hb, H[:, :, None, :]
